# revision 74
# baseline (speedup 1.0000x reference)
"""Trainium2 Bass kernel for nn_AttentionBlock (GroupNorm + single-head
channel attention + residual), distributed over 8 NeuronCores.

Problem shapes (hardcoded): x [B=16, C=512, H=32, W=32], N = H*W = 1024
tokens of C channels per batch. Weights Wq/Wk/Wv/Wp [C, C], biases [C].

  h    = GroupNorm(x; 8 groups) -> tokens [B, N, C]
  q,k,v = h @ W{q,k,v}.T + b
  attn = softmax(q k^T / sqrt(C))
  out  = (attn @ v) @ Wp.T + bp
  y    = x + out  (back in [B, C, H, W])

Sharding: data-parallel over batch, 2 batches per core, no collectives.

Per-core strategy (channels-on-partitions, [C, N] everywhere):
  - GroupNorm stats via bn_stats per partition + tiny exact-fp32
    indicator matmuls to reduce/broadcast across the 64-channel groups
    (which live on the partition axis).
  - All large matmuls run in fp8 (e4m3) with perf_mode=DoubleRow:
    operands are stored "folded" as [128, 2, free] tiles where the
    middle index j selects contraction row j*128+p, so each matmul
    contracts K=256 at 0.5 cycles/row — 4x fewer PE cycles than the
    f32 path. Weights are quantized and pre-folded on the host.
  - Attention runs transposed: ST = k^T q tiles [m=key, q]; softmax's
    exp is a scalar-engine pass straight to fp8 (logits are O(1), max
    subtraction skipped); the denominator (a column sum over the
    partition axis) comes from an ones-stationary DoubleRow matmul,
    reciprocal'd and broadcast over partitions with a rank-1 f32r
    matmul; PV evacuation multiplies it in. No PE transposes anywhere.
  - proj lands directly in [C, N]; (+ fused bias, + residual) is one
    scalar_tensor_tensor pass, then DMA out. bv is folded into the
    proj bias on host (softmax rows sum to 1).

Measured accuracy vs the fp32 reference: rel-L2 ~5e-4 (the fp8
operands carry ~3% element noise, but logits only need absolute
~1e-2 accuracy and the value path averages ~1000 terms).

This walrus build accepts at most ONE sync-wait per instruction; the
two fixups below split Tile's multi-wait instructions onto 1-wait NOPs.
"""

import numpy as np

import concourse.bass as bass
import concourse.tile as tile
from concourse import mybir
from concourse.vector_clock import ScopedClock

F32 = mybir.dt.float32
F32R = mybir.dt.float32r
F8 = mybir.dt.float8e4
ALU = mybir.AluOpType
ACTF = mybir.ActivationFunctionType
DROW = mybir.MatmulPerfMode.DoubleRow

B, C, HW = 16, 512, 1024
NCORES = 8
BPC = B // NCORES          # batches per core
CT = C // 128              # c-tiles (4)
MT = HW // 128             # key tiles (8)
NHALF = 2                  # q halves of 512
GROUPS = 8
EPS = 1e-5
SCALE = float(C) ** -0.5

_patched = False


def _patch_tile_drain():
    """Tail drain carries one wait per logical proc; split onto SP NOPs."""
    global _patched
    if _patched:
        return
    _patched = True

    def _drain_and_barrier(self, tick_clock, wait_clock):
        drain_inst = self.nc.sync.drain()
        wait_clock.add_sem_waits(
            drain_inst.ins, ScopedClock({None: tick_clock.global_clock})
        )
        si = drain_inst.ins.sync_info
        waits = list(si.on_wait) if si is not None else []
        if len(waits) > 1:
            si.on_wait = waits[:1]
            for w in waits[1:]:
                nop = self.nc.sync.nop(nofuse=True, hint="drain_wait_split")
                nop.ins.sync_info = mybir.SyncInfo(on_wait=[w], on_update=[])
        self.nc.all_engine_barrier()
        assert self.sems is not None
        popped = self.nc._tile_sem_poison_stack.pop()
        assert popped is self._sem_poison
        self.nc.clear_and_free_semaphores(list(self.sems.allocated().values()))
        self.nc.all_engine_barrier()

    tile.TileContext._drain_and_barrier = _drain_and_barrier


def _split_multi_waits(nc: bass.Bass) -> int:
    """Split every >1-wait instruction onto preceding same-engine NOPs."""
    n_split = 0
    for f in nc.m.functions:
        for bb in f.blocks:
            out = []
            changed = False
            for inst in bb.instructions:
                si = inst.sync_info
                waits = list(si.on_wait) if si is not None else []
                if len(waits) > 1:
                    changed = True
                    for w in waits[:-1]:
                        nop = mybir.InstNoOp(
                            name=f"{inst.name}-ws{n_split}",
                            engine=inst.engine,
                            bass_nofuse=True,
                            sync_info=mybir.SyncInfo(on_wait=[w], on_update=[]),
                        )
                        out.append(nop)
                        n_split += 1
                    si.on_wait = [waits[-1]]
                out.append(inst)
            if changed:
                bb.instructions[:] = out
    return n_split


def build_program(reps: int = 1) -> bass.Bass:
    """reps>1 repeats the whole per-batch pipeline (timing harness only:
    the marginal wall-clock per extra rep is the HW time of one pass)."""
    _patch_tile_drain()
    nc = bass.Bass()

    x_s = nc.declare_dram_parameter("x_s", [BPC, C, HW], F32, isOutput=False)
    # fp8 weights, host-quantized and folded: [t2, p, j, out] where the
    # contraction row is t2*256 + j*128 + p
    wq8d = nc.declare_dram_parameter("wq8", [2, 128, 2, C], F8, isOutput=False)
    wk8d = nc.declare_dram_parameter("wk8", [2, 128, 2, C], F8, isOutput=False)
    wv8d = nc.declare_dram_parameter("wv8", [2, 128, 2, C], F8, isOutput=False)
    wp8d = nc.declare_dram_parameter("wp8", [2, 128, 2, C], F8, isOutput=False)
    bvec = nc.declare_dram_parameter("bvec", [5, C], F32, isOutput=False)
    inda = nc.declare_dram_parameter("inda", [CT, 128, GROUPS], F32, isOutput=False)
    indb = nc.declare_dram_parameter("indb", [CT, GROUPS, 128], F32, isOutput=False)
    y_s = nc.declare_dram_parameter("y_s", [BPC, C, HW], F32, isOutput=True)

    with tile.TileContext(nc) as tc:
        with (
            tc.tile_pool(name="const", bufs=1) as const,
            tc.tile_pool(name="xb", bufs=2) as xpool,
            tc.tile_pool(name="acts", bufs=1) as acts,
            tc.tile_pool(name="attn", bufs=1) as attn,
            tc.tile_pool(name="ps", bufs=1, space="PSUM") as ps,
        ):
            # ---- first batch's x before the weights (DMA queue order);
            # halves so bn_stats can start on the first 512 columns early
            x_pref = []
            for t in range(CT):
                xt = xpool.tile([128, HW], F32, tag=f"x{t}")
                for sg in range(2):
                    nc.sync.dma_start(
                        out=xt[:, sg * 512:(sg + 1) * 512],
                        in_=x_s[0, t * 128:(t + 1) * 128, sg * 512:(sg + 1) * 512],
                    )
                x_pref.append(xt)

            # ---- small constants (stats + evacuations gate on these) ----
            bs = const.tile([128, 5, CT], F32, tag="bvec")
            nc.sync.dma_start(
                out=bs, in_=bvec.rearrange("v (t p) -> p v t", p=128)
            )
            bq_sb = bs[:, 0, :]    # [128, CT]
            bk_sb = bs[:, 1, :]
            bpp_sb = bs[:, 2, :]
            gam_sb = bs[:, 3, :]
            bet_sb = bs[:, 4, :]

            inda_sb = const.tile([128, CT, GROUPS], F32, tag="inda")
            nc.sync.dma_start(out=inda_sb, in_=inda.rearrange("t p g -> p t g"))
            indb_sb = const.tile([GROUPS, CT, 128], F32, tag="indb")
            nc.sync.dma_start(out=indb_sb, in_=indb.rearrange("t g p -> g t p"))

            # ---- weights (already fp8 + folded; plain DMAs) ----
            w8 = {}
            for wname, wdram in (("q", wq8d), ("k", wk8d), ("v", wv8d), ("p", wp8d)):
                for t2 in range(2):
                    wt = const.tile([128, 2, C], F8, tag=f"w{wname}{t2}",
                                    name=f"w_{wname}_{t2}")
                    nc.sync.dma_start(out=wt, in_=wdram[t2])
                    w8[(wname, t2)] = wt

            ones_f32 = const.tile([128, 2, 16], F32, tag="ones_f32")
            nc.vector.memset(ones_f32, 1.0)
            ones_f8 = const.tile([128, 2, 16], F8, tag="ones_f8")  # colsum lhsT
            nc.vector.tensor_copy(ones_f8, ones_f32)
            ones_k1f = const.tile([1, 128], F32, tag="ones_k1f")
            nc.vector.memset(ones_k1f, 1.0)
            ones_k1 = const.tile([1, 128], F32R, tag="ones_k1")  # bcast stationary
            nc.vector.tensor_copy(ones_k1, ones_k1f)
            eps8 = const.tile([GROUPS, 1], F32, tag="eps8")
            nc.vector.memset(eps8, EPS)

            def phase_a1(ib, b):
                """x, GroupNorm, q/k projections."""
                # ---- load x (slot 0 was prefetched before the weights) ----
                if ib == 0:
                    x_t = x_pref
                else:
                    x_t = []
                    for t in range(CT):
                        xt = xpool.tile([128, HW], F32, tag=f"x{t}")
                        nc.sync.dma_start(
                            out=xt, in_=x_s[b, t * 128:(t + 1) * 128, :]
                        )
                        x_t.append(xt)

                # ---- GroupNorm stats ----
                mv = acts.tile([128, CT, 2], F32, tag="mv", bufs=2)
                for t in range(CT):
                    st6 = acts.tile([128, 2, 6], F32, tag="bnst", bufs=2)
                    for sg in range(2):
                        nc.vector.bn_stats(
                            out=st6[:, sg, :], in_=x_t[t][:, sg * 512:(sg + 1) * 512]
                        )
                    nc.vector.bn_aggr(out=mv[:, t, :], in_=st6)
                # mv[:, :, 1] := var + mean^2  (= E[x^2] per partition)
                msq = acts.tile([128, CT], F32, tag="msq", bufs=2)
                nc.vector.tensor_mul(msq, mv[:, :, 0], mv[:, :, 0])
                nc.vector.tensor_add(mv[:, :, 1], mv[:, :, 1], msq)
                # group sums over the partition axis (exact fp32 matmuls)
                gsum = ps.tile([GROUPS, 2], F32, tag="mm2", bufs=4)
                for t in range(CT):
                    nc.tensor.matmul(
                        gsum[:], inda_sb[:, t, :], mv[:, t, :],
                        start=(t == 0), stop=(t == CT - 1),
                    )
                gs = acts.tile([GROUPS, 2], F32, tag="gs", bufs=2)
                nc.scalar.mul(out=gs, in_=gsum[:], mul=1.0 / 64.0)
                g2 = acts.tile([GROUPS, 1], F32, tag="g2", bufs=2)
                nc.vector.tensor_mul(g2, gs[:, 0:1], gs[:, 0:1])
                var8 = acts.tile([GROUPS, 1], F32, tag="var8", bufs=2)
                nc.vector.tensor_sub(var8, gs[:, 1:2], g2)
                stats2 = acts.tile([GROUPS, 2], F32, tag="stats2", bufs=2)
                nc.scalar.activation(
                    out=stats2[:, 1:2], in_=var8, func=ACTF.Sqrt, bias=eps8, scale=1.0
                )
                nc.vector.reciprocal(out=stats2[:, 1:2], in_=stats2[:, 1:2])
                nc.vector.tensor_copy(stats2[:, 0:1], gs[:, 0:1])
                # broadcast (mean_g, rstd_g) back to channels; h in fp8 folded
                h8 = [acts.tile([128, 2, HW], F8, tag=f"h8{t2}",
                                name=f"h8_{t2}", bufs=2) for t2 in range(2)]
                for t in range(CT):
                    pstat = ps.tile([128, 2], F32, tag="mm2", bufs=4)
                    nc.tensor.matmul(
                        pstat[:], indb_sb[:, t, :], stats2[:], start=True, stop=True
                    )
                    cstat = acts.tile([128, 2], F32, tag=f"cstat{t}", bufs=2)
                    nc.scalar.copy(out=cstat, in_=pstat[:])
                    scale_t = acts.tile([128, 1], F32, tag=f"scl{t}", bufs=2)
                    nc.vector.tensor_mul(scale_t, cstat[:, 1:2], gam_sb[:, t:t + 1])
                    tmp_t = acts.tile([128, 1], F32, tag=f"tmp{t}", bufs=2)
                    nc.vector.tensor_mul(tmp_t, cstat[:, 0:1], scale_t)
                    shift_t = acts.tile([128, 1], F32, tag=f"shf{t}", bufs=2)
                    nc.vector.tensor_sub(shift_t, bet_sb[:, t:t + 1], tmp_t)
                    nc.vector.tensor_scalar(
                        out=h8[t // 2][:, t % 2, :], in0=x_t[t],
                        scalar1=scale_t, scalar2=shift_t,
                        op0=ALU.mult, op1=ALU.add,
                    )

                # ---- q, k in fp8 folded along output channels:
                # q8[t2][p, j, n] = (Wq h + bq)[t2*256 + j*128 + p, n]
                q_f = [acts.tile([128, 2, HW], F8, tag=f"q8{t2}",
                                 name=f"q8_{t2}", bufs=2) for t2 in range(2)]
                k_f = [acts.tile([128, 2, HW], F8, tag=f"k8{t2}",
                                 name=f"k8_{t2}", bufs=2) for t2 in range(2)]
                for t in range(CT):
                    t2, j = t // 2, t % 2
                    for name, dsts, bias in (("q", q_f, bq_sb), ("k", k_f, bk_sb)):
                        pmm = ps.tile([128, HW], F32, tag="mm2", bufs=4)
                        for c2 in range(2):
                            for h in range(NHALF):
                                nc.tensor.matmul(
                                    pmm[:, h * 512:(h + 1) * 512],
                                    w8[(name, c2)][:, :, t * 128:(t + 1) * 128],
                                    h8[c2][:, :, h * 512:(h + 1) * 512],
                                    start=(c2 == 0), stop=(c2 == 1),
                                    perf_mode=DROW,
                                )
                        nc.scalar.activation(
                            out=dsts[t2][:, j, :], in_=pmm[:],
                            func=ACTF.Identity, bias=bias[:, t:t + 1],
                            scale=1.0,
                        )

                # ---- v in fp8 folded [m, d]: v8[mp][p, j, d] = v[mp*256+j*128+p, d]
                v_f = [acts.tile([128, 2, 512], F8, tag=f"v8{mp}",
                                 name=f"v8_{mp}", bufs=2) for mp in range(MT // 2)]
                for mp in range(MT // 2):
                    pv = ps.tile([128, 2, 512], F32, tag="mm2", bufs=4)
                    for j in range(2):
                        m = 2 * mp + j
                        for c2 in range(2):
                            nc.tensor.matmul(
                                pv[:, j, :],
                                h8[c2][:, :, m * 128:(m + 1) * 128],
                                w8[("v", c2)][:],
                                start=(c2 == 0), stop=(c2 == 1),
                                perf_mode=DROW,
                            )
                    nc.vector.tensor_copy(v_f[mp][:], pv[:])

                return dict(x_t=x_t, h8=h8, q_f=q_f, k_f=k_f, v_f=v_f, b=b)

            def phase_a2(st):
                """S+exp+colsum, v, 1/colsum."""
                x_t, h8, q_f, k_f, v_f, b = (
                    st["x_t"], st["h8"], st["q_f"], st["k_f"], st["v_f"], st["b"])
                # ---- S^T for all m, both halves (stationary k8 shared) ----
                pt_f = [attn.tile([128, 2, HW], F8, tag=f"pt8{mp}",
                                  name=f"pt8_{mp}", bufs=2)
                        for mp in range(MT // 2)]
                cs = ps.tile([1, HW], F32, tag="mm2", bufs=4, name="cs")
                for m in range(MT):
                    pst = ps.tile([128, HW], F32, tag="mm2", bufs=4, name="pst")
                    for t2 in range(2):
                        for h in range(NHALF):
                            nc.tensor.matmul(
                                pst[:, h * 512:(h + 1) * 512],
                                k_f[t2][:, :, m * 128:(m + 1) * 128],
                                q_f[t2][:, :, h * 512:(h + 1) * 512],
                                start=(t2 == 0), stop=(t2 == 1),
                                perf_mode=DROW,
                            )
                    nc.scalar.activation(
                        out=pt_f[m // 2][:, m % 2, :], in_=pst[:],
                        func=ACTF.Exp, scale=SCALE,
                    )
                    if m % 2 == 1:
                        # colsum chunk as soon as this mp's exp is queued
                        mp = m // 2
                        for h in range(NHALF):
                            nc.tensor.matmul(
                                cs[:, h * 512:(h + 1) * 512],
                                ones_f8[:, :, 0:1],
                                pt_f[mp][:, :, h * 512:(h + 1) * 512],
                                start=(mp == 0), stop=(mp == MT // 2 - 1),
                                perf_mode=DROW,
                            )

                # 1/colsum row straight to f32r (frees the cs PSUM slot)
                rrow_r = attn.tile([1, HW], F32R, tag="rrow_r", bufs=2)
                with nc.allow_low_precision(reason="f32r reciprocal feeds a rank-1 broadcast matmul"):
                    nc.vector.reciprocal(out=rrow_r, in_=cs[:])
                st.update(v_f=v_f, pt_f=pt_f, rrow_r=rrow_r)
                return st

            def phase_b1(st):
                """broadcast, PV + normalize."""
                b, x_t, v_f, pt_f = st["b"], st["x_t"], st["v_f"], st["pt_f"]
                pbc = ps.tile([128, HW], F32, tag="mm2", bufs=4)
                for h in range(NHALF):
                    nc.tensor.matmul(
                        pbc[:, h * 512:(h + 1) * 512], ones_k1[:],
                        st["rrow_r"][:, h * 512:(h + 1) * 512],
                        start=True, stop=True,
                    )

                # PV: outT[d, q] over key pairs, normalized into fp8 folded
                bc = attn.tile([128, HW], F32, tag="bc", bufs=2)
                nc.scalar.copy(out=bc, in_=pbc[:])
                oc8 = [attn.tile([128, 2, HW], F8, tag=f"oc8{d2}",
                                 name=f"oc8_{d2}", bufs=2) for d2 in range(2)]
                for d in range(CT):
                    po = ps.tile([128, HW], F32, tag="mm2", bufs=4)
                    for mp in range(MT // 2):
                        for h in range(NHALF):
                            nc.tensor.matmul(
                                po[:, h * 512:(h + 1) * 512],
                                v_f[mp][:, :, d * 128:(d + 1) * 128],
                                pt_f[mp][:, :, h * 512:(h + 1) * 512],
                                start=(mp == 0), stop=(mp == MT // 2 - 1),
                                perf_mode=DROW,
                            )
                    nc.vector.tensor_mul(oc8[d // 2][:, d % 2, :], po[:], bc)

                st.update(oc8=oc8)
                return st

            def phase_b2(st):
                """proj + fused bias + residual, y DMA."""
                b, x_t, oc8 = st["b"], st["x_t"], st["oc8"]
                for e in range(CT):
                    pp = ps.tile([128, HW], F32, tag="mm2", bufs=4)
                    for d2 in range(2):
                        for h in range(NHALF):
                            nc.tensor.matmul(
                                pp[:, h * 512:(h + 1) * 512],
                                w8[("p", d2)][:, :, e * 128:(e + 1) * 128],
                                oc8[d2][:, :, h * 512:(h + 1) * 512],
                                start=(d2 == 0), stop=(d2 == 1),
                                perf_mode=DROW,
                            )
                    ye = acts.tile([128, HW], F32, tag=f"y{e}", bufs=2)
                    nc.vector.scalar_tensor_tensor(
                        out=ye, in0=pp[:], scalar=bpp_sb[:, e:e + 1],
                        in1=x_t[e][:], op0=ALU.add, op1=ALU.add,
                    )
                    nc.sync.dma_start(
                        out=y_s[b, e * 128:(e + 1) * 128, :], in_=ye,
                    )

            # fine-grained software pipeline over the flat batch list:
            # B-phase DVE evacuations of batch i-1 are emitted interleaved
            # with the A-phase (scalar-engine-bound) work of batch i
            seq = [b for _ in range(reps) for b in range(BPC)]
            prev = None
            for i, b in enumerate(seq):
                st = phase_a1(i, b)
                bst = phase_b1(prev) if prev is not None else None
                st = phase_a2(st)
                if bst is not None:
                    phase_b2(bst)
                prev = st
            phase_b2(phase_b1(prev))

    _split_multi_waits(nc)
    return nc


_program_cache = {}


def _get_program(reps: int = 1) -> bass.Bass:
    if reps not in _program_cache:
        _program_cache[reps] = build_program(reps)
    return _program_cache[reps]


def _fold_fp8(wT: np.ndarray) -> np.ndarray:
    """[K, M] -> folded fp8 [2, 128, 2, M]: arr[t2, p, j] = wT[t2*256+j*128+p]."""
    f8 = mybir.dt.np(F8)
    return np.ascontiguousarray(
        wT.reshape(2, 2, 128, wT.shape[1]).transpose(0, 2, 1, 3)
    ).astype(f8)


def make_in_maps(**inputs) -> list[dict]:
    x = np.ascontiguousarray(np.asarray(inputs["x"], dtype=np.float32))
    Wq = np.asarray(inputs["Wq"], np.float32)
    Wk = np.asarray(inputs["Wk"], np.float32)
    Wv = np.asarray(inputs["Wv"], np.float32)
    Wp = np.asarray(inputs["Wp"], np.float32)
    bq = np.asarray(inputs["bq"], np.float32)
    bk = np.asarray(inputs["bk"], np.float32)
    bv = np.asarray(inputs["bv"], np.float32)
    bp = np.asarray(inputs["bp"], np.float32)
    gamma = np.asarray(inputs["gn_gamma"], np.float32)
    beta = np.asarray(inputs["gn_beta"], np.float32)

    # fp8, folded for DoubleRow (contraction over the first axis)
    wq8 = _fold_fp8(np.ascontiguousarray(Wq.T))
    wk8 = _fold_fp8(np.ascontiguousarray(Wk.T))
    wv8 = _fold_fp8(np.ascontiguousarray(Wv.T))
    wp8 = _fold_fp8(np.ascontiguousarray(Wp.T))
    # softmax rows sum to 1, so  attn @ (v + bv) @ Wp.T + bp
    #   = attn @ v @ Wp.T + (Wp bv + bp)
    bpp = (bp + Wp @ bv).astype(np.float32)
    bvec = np.ascontiguousarray(np.stack([bq, bk, bpp, gamma, beta]))

    # indicator matrices for group reduce/broadcast over the partition axis
    inda = np.zeros((CT, 128, GROUPS), np.float32)
    indb = np.zeros((CT, GROUPS, 128), np.float32)
    for t in range(CT):
        for p in range(128):
            g = (t * 128 + p) // (C // GROUPS)
            inda[t, p, g] = 1.0
            indb[t, g, p] = 1.0

    xr = x.reshape(B, C, HW)
    shared = dict(wq8=wq8, wk8=wk8, wv8=wv8, wp8=wp8, bvec=bvec,
                  inda=inda, indb=indb)
    return [
        dict(shared, x_s=np.ascontiguousarray(xr[i * BPC:(i + 1) * BPC]))
        for i in range(NCORES)
    ]


def kernel(**inputs) -> np.ndarray:
    from concourse.bass_utils import run_bass_kernel_spmd

    nc = _get_program()
    in_maps = make_in_maps(**inputs)
    res = run_bass_kernel_spmd(nc, in_maps, list(range(NCORES)))
    y = np.concatenate([res.results[i]["y_s"] for i in range(NCORES)], axis=0)
    return y.reshape(B, C, 32, 32).astype(np.float32)
